# revision 50
# baseline (speedup 1.0000x reference)
"""Additive (Bahdanau) attention kernel for Trainium2, SPMD over 8 NeuronCores.

Math (per batch b):
    q      = h[b] @ W_h                          # [A]
    K      = enc_out[b] @ W_s                    # [S, A]
    energy = tanh(q + K)                         # [S, A]
    score  = energy @ v                          # [S]
    score  = where(mask, -1e9, score)
    attn   = softmax(score)                      # [S]
    ctx    = attn @ enc_out[b]                   # [E]
returns (ctx [B,E], attn [B,S])

Sharding: data-parallel over batch B=32 -> 4 batches per core, 8 cores.
No collectives needed (each core's batches are independent).

Per-core schedule (transposed orientation, S-tiles of 512):
  - DMA enc chunks [128s, 512e] f32 (4 per s-tile, separate DMAs so many
    HW-DGE queues run concurrently), cast to bf16 on DVE
  - TensorE-transpose 128x128 blocks -> PSUM, copy to SBUF encT
    [128e, 4j, 512s] bf16
  - K^T[a,s] = sum_j Ws_bf[ej,a]-block^T @ encT_j  (TensorE bf16, PSUM f32)
  - energyT = tanh(K^T + q) on ScalarE, q fused as per-partition bias
  - score row [1,512] = sum_m v_m^T @ energyT_m + maskneg row (K=1 MM),
    all accumulated in one PSUM bank
  - w = exp(psS) on ScalarE straight from PSUM; sum(w) via accum_out
  - w row TensorE-transposed to columns; ctx[1,512] += w_col^T @ enc_chunk
    accumulated in PSUM across the whole batch
  - per batch: l = sum of exp-sums, ctx = psCtx/l, attn = w/l (DVE), DMA out

Notes:
  - softmax computed without max-subtraction: |score| <= sum|v_a| ~ 18, so
    exp() cannot overflow f32 and masked lanes hit exp(-1e9) == 0.
  - mask is pre-scaled on the host to bf16 (-1e9 * mask) and added to the
    scores via a K=1 ones matmul into the score PSUM accumulation group.
  - engine ops may only start at partitions 0/32/64/96, so all per-batch
    row vectors (scores/weights) live on partition 0.
  - all big matmuls run in bf16 (1 cycle/row); plain f32 is 4 cycles/row
    and float32r has producer-rounding constraints the BIR verifier
    rejects for copy/DMA-produced operands.
  - tensor_tensor_reduce crashes this runtime (device unrecoverable), so
    the context reduction uses TensorE matmuls instead.
  - no DRAM scratch round-trips: Tile does not order DMA-out vs DMA-in on
    the same DRAM tensor across queues (races on hardware).
"""

import sys
import numpy as np

sys.path.insert(0, "/opt/trn_rl_repo")

B, S, ENC, DEC, ATTN = 32, 4096, 512, 512, 512
NCORES = 8
BPC = B // NCORES          # batches per core
ST = 512                   # s-tile size
NST = S // ST              # s-tiles per batch
NEG = -1.0e9

_CACHE = {}


def _build(loop_n=1):
    key = ("nc", loop_n)
    if key in _CACHE:
        return _CACHE[key]

    from contextlib import ExitStack
    from concourse import bacc, tile, mybir

    f32 = mybir.dt.float32

    nc = bacc.Bacc("TRN2", target_bir_lowering=False, debug=False,
                   num_devices=NCORES)

    h_d = nc.dram_tensor("h", [BPC, DEC], f32, kind="ExternalInput").ap()
    enc_d = nc.dram_tensor("enc_out", [BPC, S, ENC], f32,
                           kind="ExternalInput").ap()
    mask_d = nc.dram_tensor("mask", [BPC, S], mybir.dt.bfloat16,
                            kind="ExternalInput").ap()
    wh_d = nc.dram_tensor("W_h", [DEC, ATTN], f32, kind="ExternalInput").ap()
    ws_d = nc.dram_tensor("W_s", [ENC, ATTN], f32, kind="ExternalInput").ap()
    v_d = nc.dram_tensor("v", [ATTN], f32, kind="ExternalInput").ap()
    ctx_d = nc.dram_tensor("ctx", [BPC, ENC], f32, kind="ExternalOutput").ap()
    attn_d = nc.dram_tensor("attn", [BPC, S], f32, kind="ExternalOutput").ap()

    with tile.TileContext(nc) as tc, ExitStack() as st_:
        _kernel(tc, st_, mybir,
                h_d, enc_d, mask_d, wh_d, ws_d, v_d, ctx_d, attn_d, loop_n)

    nc.compile()
    _CACHE[key] = nc
    return nc


def _kernel(tc, ctx_stack, mybir, h_d, enc_d, mask_d, wh_d, ws_d, v_d,
            ctx_d, attn_d, loop_n=1):
    from contextlib import ExitStack as _ES

    nc = tc.nc
    f32 = mybir.dt.float32
    f32r = mybir.dt.float32r
    bf16 = mybir.dt.bfloat16
    i32 = mybir.dt.int32
    Alu = mybir.AluOpType
    Act = mybir.ActivationFunctionType

    const = ctx_stack.enter_context(tc.tile_pool(name="const", bufs=1))
    persist = ctx_stack.enter_context(tc.tile_pool(name="persist", bufs=1))

    # --- identity matrix for TensorE transposes ---
    iota = const.tile([128, 128], i32)
    nc.gpsimd.iota(iota[:], pattern=[[-1, 128]], base=0, channel_multiplier=1)
    ident = const.tile([128, 128], bf16)
    nc.vector.tensor_scalar(out=ident[:], in0=iota[:], scalar1=0,
                            scalar2=None, op0=Alu.is_equal)
    identf = const.tile([128, 128], f32)
    nc.vector.tensor_scalar(out=identf[:], in0=iota[:], scalar1=0,
                            scalar2=None, op0=Alu.is_equal)
    onec = const.tile([1, 1], bf16)
    nc.vector.memset(onec[:], 1.0)

    # --- load weights: W_s as 4 e-chunk tiles [128, 512], cast to bf16 ---
    ws = []
    for j in range(4):
        t32 = const.tile([128, 512], f32, name=f"wsf{j}")
        nc.sync.dma_start(out=t32[:], in_=ws_d[128 * j:128 * (j + 1), :])
        t = const.tile([128, 512], bf16, name=f"ws{j}")
        nc.vector.tensor_copy(out=t[:], in_=t32[:])
        ws.append(t)

    # --- h, v ---
    h_sb = const.tile([BPC, DEC], f32)
    nc.sync.dma_start(out=h_sb[:], in_=h_d[:, :])
    # v as columns [128, 4] (a = 128*m + p), cast to bf16 for the vdot MM
    vtf = const.tile([128, 4], f32)
    nc.sync.dma_start(out=vtf[:], in_=v_d.rearrange("(m p) -> p m", p=128))
    vt = const.tile([128, 4], bf16)
    nc.vector.tensor_copy(out=vt[:], in_=vtf[:])

    # --- q^T = (h @ W_h)^T as 4 a-chunk tiles [128, BPC] ---
    qt = []
    with tc.tile_pool(name="setup_ps", bufs=2, space="PSUM") as setup_ps, \
            tc.tile_pool(name="setup_sb", bufs=1) as setup_sb:
        wh = []
        for j in range(4):
            t = setup_sb.tile([128, 512], f32, name=f"wh{j}")
            nc.sync.dma_start(out=t[:], in_=wh_d[128 * j:128 * (j + 1), :])
            wh.append(t)
        ht = []
        for j in range(4):
            ps = setup_ps.tile([128, BPC], f32, tag="psq", name=f"psht{j}")
            nc.tensor.transpose(ps[:], h_sb[:, 128 * j:128 * (j + 1)],
                                identf[0:BPC, 0:BPC])
            t = setup_sb.tile([128, BPC], f32, name=f"ht{j}")
            nc.vector.tensor_copy(out=t[:], in_=ps[:])
            ht.append(t)
        for m in range(4):
            ps = setup_ps.tile([128, BPC], f32, tag="psq", name=f"psqt{m}")
            for j in range(4):
                nc.tensor.matmul(ps[:],
                                 lhsT=wh[j][:, 128 * m:128 * (m + 1)],
                                 rhs=ht[j][:],
                                 start=(j == 0), stop=(j == 3))
            t = const.tile([128, BPC], f32, name=f"qt{m}")
            nc.vector.tensor_copy(out=t[:], in_=ps[:])
            qt.append(t)

    # --- persistent accumulators ---
    ws_sums = persist.tile([1, BPC * NST], f32)          # exp-sum per s-tile
    rl_row = persist.tile([1, BPC], f32)                 # 1/l per batch
    w_allB = persist.tile([1, BPC * S], bf16)            # unnormalized w rows

    loop_stack = _ES()
    main = loop_stack.enter_context(tc.tile_pool(name="main", bufs=4))
    chunkp = loop_stack.enter_context(tc.tile_pool(name="chunkp", bufs=16))
    small = loop_stack.enter_context(tc.tile_pool(name="small", bufs=3))
    bpool = loop_stack.enter_context(tc.tile_pool(name="bpool", bufs=2))
    psT_pool = loop_stack.enter_context(
        tc.tile_pool(name="psT", bufs=2, space="PSUM"))
    psK_pool = loop_stack.enter_context(
        tc.tile_pool(name="psK", bufs=2, space="PSUM"))
    psS_pool = loop_stack.enter_context(
        tc.tile_pool(name="psS", bufs=2, space="PSUM"))
    psW_pool = loop_stack.enter_context(
        tc.tile_pool(name="psW", bufs=1, space="PSUM"))
    psC_pool = loop_stack.enter_context(
        tc.tile_pool(name="psC", bufs=1, space="PSUM"))

    def batch_body(b):
        # per-batch: mask row (pre-scaled -1e9*mask); w rows live in w_allB
        mask_b = bpool.tile([1, S], bf16, tag="maskb", name=f"maskb_{b}")
        nc.sync.dma_start(out=mask_b[:], in_=mask_d[b:b + 1, :])
        w_all = w_allB[0:1, b * S:(b + 1) * S]
        psCtx = psC_pool.tile([1, 512], f32, tag="psC", name=f"psC_{b}")

        for sti in range(NST):
            s0 = ST * sti
            # load 4 chunks [128s, 512e] (separate DMAs -> more queue overlap)
            enc_bf = []
            for cc in range(4):
                ef = chunkp.tile([128, 512], f32, tag="enc",
                                 name=f"enc_{b}_{sti}_{cc}")
                nc.sync.dma_start(
                    out=ef[:],
                    in_=enc_d[b, s0 + 128 * cc:s0 + 128 * (cc + 1), :])
                eb = chunkp.tile([128, 512], bf16, tag="encbf",
                                 name=f"encbf_{b}_{sti}_{cc}")
                nc.vector.tensor_copy(out=eb[:], in_=ef[:])
                enc_bf.append(eb)

            # transpose -> encT [128e, 4j, 512s] bf16
            encT = main.tile([128, 4, 512], bf16, tag="encT",
                             name=f"encT_{b}_{sti}")
            for cc in range(4):
                psT = psT_pool.tile([128, 512], bf16, tag="psT",
                                    name=f"psT_{b}_{sti}_{cc}")
                for j in range(4):
                    nc.tensor.transpose(psT[:, 128 * j:128 * (j + 1)],
                                        enc_bf[cc][:, 128 * j:128 * (j + 1)],
                                        ident[:])
                nc.vector.tensor_copy(
                    out=encT[:, :, 128 * cc:128 * cc + 128],
                    in_=psT[:].rearrange("p (j s) -> p j s", j=4))

            # K^T chunks + tanh(. + q) -> energyT [128a, 4m, 512s] bf16
            energyT = main.tile([128, 4, 512], bf16, tag="en",
                                name=f"en_{b}_{sti}")
            for m in range(4):
                psK = psK_pool.tile([128, 512], f32, tag="psK",
                                    name=f"psK_{b}_{sti}_{m}")
                for j in range(4):
                    nc.tensor.matmul(psK[:],
                                     lhsT=ws[j][:, 128 * m:128 * (m + 1)],
                                     rhs=_enc_t_j(encT, j),
                                     start=(j == 0), stop=(j == 3))
                nc.scalar.activation(energyT[:, m, :], psK[:], Act.Tanh,
                                     bias=qt[m][:, b:b + 1])

            # score row [1, 512] = v . energyT + maskneg
            psS = psS_pool.tile([1, 512], f32, tag="psS",
                                name=f"psS_{b}_{sti}")
            for m in range(4):
                nc.tensor.matmul(psS[:], lhsT=vt[:, m:m + 1],
                                 rhs=energyT[:, m, :],
                                 start=(m == 0), stop=False)
            nc.tensor.matmul(psS[:], lhsT=onec[:],
                             rhs=mask_b[0:1, s0:s0 + ST],
                             start=False, stop=True)

            # w = exp(score); accumulate sum(w) into ws_sums
            col = b * NST + sti
            nc.scalar.activation(w_all[0:1, s0:s0 + ST], psS[:], Act.Exp,
                                 accum_out=ws_sums[:, col:col + 1])

            # transpose w row to columns for the ctx matmul (PSUM writes
            # need 4-byte alignment, so bf16 columns go to even offsets)
            psW = psW_pool.tile([128, 8], bf16, tag="psW",
                                name=f"psW_{b}_{sti}")
            for cc in range(4):
                nc.tensor.transpose(
                    psW[:, 2 * cc:2 * cc + 1],
                    w_all[0:1, s0 + 128 * cc:s0 + 128 * (cc + 1)],
                    ident[0:1, 0:1])
            wcol = small.tile([128, 4], bf16, tag="wcol",
                              name=f"wcol_{b}_{sti}")
            nc.vector.tensor_copy(out=wcol[:], in_=psW[:, 0:8:2])

            # ctx += w_chunk^T @ enc_chunk  (accumulate over whole batch)
            for cc in range(4):
                nc.tensor.matmul(psCtx[:], lhsT=wcol[:, cc:cc + 1],
                                 rhs=enc_bf[cc][:],
                                 start=(sti == 0 and cc == 0),
                                 stop=(sti == NST - 1 and cc == 3))

        # normalizer for this batch, then ctx = psCtx / l
        lb = small.tile([1, 1], f32, tag="lb", name=f"lb_{b}")
        nc.vector.tensor_reduce(out=lb[:],
                                in_=ws_sums[:, b * NST:(b + 1) * NST],
                                axis=mybir.AxisListType.X, op=Alu.add)
        nc.vector.reciprocal(rl_row[:, b:b + 1], lb[:])
        ctx_row = small.tile([1, 512], f32, tag="ctxrow", name=f"ctxr_{b}")
        nc.vector.tensor_scalar(out=ctx_row[:], in0=psCtx[:],
                                scalar1=rl_row[0:1, b:b + 1], scalar2=None,
                                op0=Alu.mult)
        nc.sync.dma_start(out=ctx_d[b:b + 1, :], in_=ctx_row[:])

        # attn rows for this batch: attn = w / l (all on partition 0)
        attn_row = bpool.tile([1, S], f32, tag="attnrow", name=f"attnr_{b}")
        nc.vector.tensor_scalar(out=attn_row[:], in0=w_all,
                                scalar1=rl_row[0:1, b:b + 1], scalar2=None,
                                op0=Alu.mult)
        nc.sync.dma_start(out=attn_d[b:b + 1, :], in_=attn_row[:])

    def main_body():
        for b in range(BPC):
            batch_body(b)

    if loop_n > 1:
        with tc.For_i(0, loop_n, 1):
            main_body()
    else:
        main_body()

    loop_stack.close()


def _enc_t_j(encT, j):
    """encT_j rhs [128e, 512s] for matmul: plane j of [128, 4, 512]."""
    return encT[:, j, :]


def _make_in_maps(h, enc_out, mask, W_h, W_s, v):
    import ml_dtypes

    h = np.ascontiguousarray(h, dtype=np.float32)
    enc_out = np.ascontiguousarray(enc_out, dtype=np.float32)
    maskneg = np.where(np.asarray(mask), np.float32(NEG), np.float32(0.0))
    maskneg = np.ascontiguousarray(maskneg.astype(ml_dtypes.bfloat16))
    W_h = np.ascontiguousarray(W_h, dtype=np.float32)
    W_s = np.ascontiguousarray(W_s, dtype=np.float32)
    v = np.ascontiguousarray(v, dtype=np.float32)

    in_maps = []
    for i in range(NCORES):
        lo, hi = BPC * i, BPC * (i + 1)
        in_maps.append({
            "h": h[lo:hi],
            "enc_out": enc_out[lo:hi],
            "mask": maskneg[lo:hi],
            "W_h": W_h,
            "W_s": W_s,
            "v": v,
        })
    return in_maps


def run(in_maps, loop_n=1):
    from concourse.bass_utils import run_bass_kernel_spmd

    nc = _build(loop_n)
    res = run_bass_kernel_spmd(nc, in_maps, core_ids=list(range(NCORES)))
    _CACHE["last_result"] = res
    return res


def kernel(h, enc_out, mask, W_h, W_s, v):
    in_maps = _make_in_maps(h, enc_out, mask, W_h, W_s, v)
    res = run(in_maps, 1)
    ctx = np.concatenate([res.results[i]["ctx"] for i in range(NCORES)], axis=0)
    attn = np.concatenate([res.results[i]["attn"] for i in range(NCORES)],
                          axis=0)
    return ctx, attn
